# revision 28
# baseline (speedup 1.0000x reference)
"""Trainium2 Bass kernel for DeepSeek-style attention (B=2, S=2048, H=2048,
NH=16, NKV=4, HD=128, repeat_interleave GQA quirk, RoPE, causal mask).

Sharding: 8 cores = 2 (batch) x 4 (kv-head group).  Each core computes
q/k/v projections for its kv group (4 q heads share 1 kv head), RoPE,
attention, and a partial o_proj against its 512-column slice of Wo.
The 4 partial o_proj outputs per batch are summed on the host.

All layouts are prepared host-side:
  xT   [H, S]        x transposed (contraction dim major), bf16
  wqT  [H, 512]      Wq slice transposed, bf16
  wkT  [H, 128], wvT [H, 128]
  woT  [512, H]      Wo slice transposed (d-major), bf16
  cosT [128, S]      rope cos, head-dim major, bf16
  sinP [128, S]      rope sin, sign-folded + pre-rotated by 64, bf16
  maskb [128, nblk, 128]  unique "mixed" mask blocks, transposed, x sqrt(HD)

Device algorithm highlights:
  * scores are computed transposed ([k, q] layout) so the exp'd probs tile
    is directly the stationary operand of the P@V matmul - no transposes.
  * softmax denominator comes free from a ones-column appended to V
    (contraction over k accumulates sum(exp) in psum column 128).
  * no max-subtraction in softmax (scores are O(5); exp is safe in f32,
    and softmax is shift-invariant so results match the reference).
  * mask blocks are classified host-side: all-zero blocks add nothing,
    blocks entirely < -30 are skipped (exp underflows to 0 relative to
    in-row survivors), mixed blocks get a DVE add of the stored block.
"""

import math
from contextlib import ExitStack

import ml_dtypes
import numpy as np

import concourse.bass as bass
import concourse.mybir as mybir
import concourse.tile as tile
from concourse import bacc
from concourse.bass_utils import run_bass_kernel_spmd
from concourse.masks import make_identity

B, S, H = 2, 2048, 2048
NH, NKV, HD = 16, 4, 128
P = 128
NB = S // P          # 16 s blocks
HC = H // P          # 16 h chunks
HPG = NH // NKV      # 4 q heads per core
QCH = 512            # q chunk width
NQC = S // QCH       # 4 q chunks
SCALE = 1.0 / math.sqrt(HD)
SQRT_HD = math.sqrt(HD)
F32 = mybir.dt.float32
BF16 = mybir.dt.bfloat16
N_CORES = 8


def _classify_mask(mask):
    """Per 128x128 block: 'zero' (no-op), 'skip' (fully masked), or an index
    into the list of unique transposed/pre-scaled mask blocks."""
    kinds = [[None] * NB for _ in range(NB)]
    uniq, blocks = {}, []
    for qi in range(NB):
        for ki in range(NB):
            sub = mask[qi * P:(qi + 1) * P, ki * P:(ki + 1) * P]
            if not sub.any():
                kinds[qi][ki] = "zero"
            elif sub.max() < -30.0:
                kinds[qi][ki] = "skip"
            else:
                blkT = np.ascontiguousarray(sub.T * SQRT_HD, dtype=np.float32)
                key = blkT.tobytes()
                if key not in uniq:
                    uniq[key] = len(blocks)
                    blocks.append(blkT)
                kinds[qi][ki] = uniq[key]
    return kinds, blocks


def _build_program(kinds, n_blocks):
    nc = bacc.Bacc()
    xT = nc.declare_dram_parameter("xT", [H, S], BF16, isOutput=False)
    wqT = nc.declare_dram_parameter("wqT", [H, HPG * HD], BF16, isOutput=False)
    wkT = nc.declare_dram_parameter("wkT", [H, HD], BF16, isOutput=False)
    wvT = nc.declare_dram_parameter("wvT", [H, HD], BF16, isOutput=False)
    woT = nc.declare_dram_parameter("woT", [HPG * HD, H], BF16, isOutput=False)
    cosT = nc.declare_dram_parameter("cosT", [HD, S], BF16, isOutput=False)
    sinP = nc.declare_dram_parameter("sinP", [HD, S], BF16, isOutput=False)
    perm = nc.declare_dram_parameter("perm", [P, P], BF16, isOutput=False)
    maskb = None
    if n_blocks:
        maskb = nc.declare_dram_parameter("maskb", [P, n_blocks, P], F32,
                                          isOutput=False)
    out = nc.declare_dram_parameter("out", [S, H], F32, isOutput=True)

    with tile.TileContext(nc) as tc, ExitStack() as ctx:
        consts = ctx.enter_context(tc.tile_pool(name="consts", bufs=1))
        xT_sb = consts.tile([P, HC, S], BF16, tag="xT")
        wqT_sb = consts.tile([P, HC, HPG * HD], BF16, tag="wqT")
        wkT_sb = consts.tile([P, HC, HD], BF16, tag="wkT")
        wvT_sb = consts.tile([P, HC, HD], BF16, tag="wvT")
        woT_sb = consts.tile([P, HPG, H], BF16, tag="woT")
        cos_sb = consts.tile([P, S], BF16, tag="cos")
        sin_sb = consts.tile([P, S], BF16, tag="sin")
        ident = consts.tile([P, P], BF16, tag="ident")
        make_identity(nc, ident)

        # DMA issue order = consumption order: rope constants first (the
        # first RoPE stalls the whole PE pipeline if cos/sin land last),
        # then k/v weights, then x interleaved with q weights; o_proj
        # weights and mask blocks are needed latest.
        perm_sb = consts.tile([P, P], BF16, tag="perm")
        nc.sync.dma_start(out=perm_sb[:], in_=perm[:])
        for hc in range(HC):
            nc.sync.dma_start(out=wkT_sb[:, hc, :], in_=wkT[hc * P:(hc + 1) * P, :])
            nc.sync.dma_start(out=wvT_sb[:, hc, :], in_=wvT[hc * P:(hc + 1) * P, :])
            nc.sync.dma_start(out=xT_sb[:, hc, :], in_=xT[hc * P:(hc + 1) * P, :])
            nc.sync.dma_start(out=wqT_sb[:, hc, :], in_=wqT[hc * P:(hc + 1) * P, :])
            if hc in (2, 5, 8, 11):
                # rope constants, spread out so no single DMA bubble
                # starves the chunk-paced projection matmuls
                j = {2: 0, 5: 1, 8: 2, 11: 3}[hc]
                sl = slice(j * QCH, (j + 1) * QCH)
                nc.sync.dma_start(out=cos_sb[:, sl], in_=cosT[:, sl])
                nc.sync.dma_start(out=sin_sb[:, sl], in_=sinP[:, sl])
        mask_sb = None
        if n_blocks:
            mask_sb = consts.tile([P, n_blocks, P], F32, tag="maskb")
            nc.sync.dma_start(out=mask_sb[:], in_=maskb[:])
        for g in range(HPG):
            nc.sync.dma_start(out=woT_sb[:, g, :], in_=woT[g * P:(g + 1) * P, :])

        # persistent activation buffers
        qrot_sb = consts.tile([P, HPG, S], BF16, tag="qrot")
        krot_sb = consts.tile([P, S], BF16, tag="krot")
        vaug_sb = consts.tile([P, NB, HD + 1], BF16, tag="vaug")

        rope_tmp = ctx.enter_context(tc.tile_pool(name="rope_tmp", bufs=3))

        with tc.tile_pool(name="proj_ps", bufs=8, space="PSUM") as proj_ps:

            def rope_quarter(ps, dst, sq):
                """dst = ps * cos + rot64(ps * sinP), cast to bf16.  The
                partition rotation by 64 is a PE matmul with a permutation
                matrix (cross-partition moves aren't a DVE thing)."""
                sl = slice(sq * QCH, (sq + 1) * QCH)
                t1 = rope_tmp.tile([P, QCH], F32, tag="t1")
                u = rope_tmp.tile([P, QCH], BF16, tag="u")
                nc.vector.tensor_mul(t1[:], ps[:], cos_sb[:, sl])
                nc.vector.tensor_mul(u[:], ps[:], sin_sb[:, sl])
                us_ps = proj_ps.tile([P, QCH], F32, tag="ps")
                nc.tensor.matmul(us_ps[:], perm_sb[:], u[:],
                                 start=True, stop=True)
                nc.vector.tensor_add(dst, t1[:], us_ps[:])

            # k + v projections, h-chunk-major: the PE consumes xT chunks in
            # DMA arrival order (no head-of-line blocking on late chunks).
            # k/v outputs are d-major; 8 accumulators = all 8 psum banks.
            vT_sb = consts.tile([P, S], BF16, tag="vT")
            kps = [proj_ps.tile([P, QCH], F32, tag="ps", name=f"kps{i}")
                   for i in range(NQC)]
            vps = [proj_ps.tile([P, QCH], F32, tag="ps", name=f"vps{i}")
                   for i in range(NQC)]
            for hc in range(HC):
                for sq in range(NQC):
                    nc.tensor.matmul(
                        kps[sq][:], wkT_sb[:, hc, :],
                        xT_sb[:, hc, sq * QCH:(sq + 1) * QCH],
                        start=(hc == 0), stop=(hc == HC - 1))
                for sq in range(NQC):
                    nc.tensor.matmul(
                        vps[sq][:], wvT_sb[:, hc, :],
                        xT_sb[:, hc, sq * QCH:(sq + 1) * QCH],
                        start=(hc == 0), stop=(hc == HC - 1))
            for sq in range(NQC):
                nc.scalar.copy(out=vT_sb[:, sq * QCH:(sq + 1) * QCH],
                               in_=vps[sq][:])
            for sq in range(NQC):
                rope_quarter(kps[sq], krot_sb[:, sq * QCH:(sq + 1) * QCH], sq)
            # v: PE-transpose each 128-block to the s-major layout PV needs
            for si in range(NB):
                vt = proj_ps.tile([P, P], BF16, tag="ps")
                nc.tensor.transpose(vt[:], vT_sb[:, si * P:(si + 1) * P], ident[:])
                nc.scalar.copy(out=vaug_sb[:, si, 0:HD], in_=vt[:])
                nc.vector.memset(vaug_sb[:, si, HD:HD + 1], 1.0)
            # q projections + rope, per head (xT resident by now)
            for h in range(HPG):
                qps = [proj_ps.tile([P, QCH], F32, tag="ps", name=f"qps{i}")
                       for i in range(NQC)]
                for hc in range(HC):
                    for sq in range(NQC):
                        nc.tensor.matmul(
                            qps[sq][:], wqT_sb[:, hc, h * HD:(h + 1) * HD],
                            xT_sb[:, hc, sq * QCH:(sq + 1) * QCH],
                            start=(hc == 0), stop=(hc == HC - 1))
                for sq in range(NQC):
                    rope_quarter(qps[sq], qrot_sb[:, h, sq * QCH:(sq + 1) * QCH],
                                 sq)

        # attention pools (reuse banks freed by proj_ps)
        qk_ps = ctx.enter_context(tc.tile_pool(name="qk_ps", bufs=2, space="PSUM"))
        pv_ps = ctx.enter_context(tc.tile_pool(name="pv_ps", bufs=2, space="PSUM"))
        probs_pool = ctx.enter_context(tc.tile_pool(name="probs", bufs=21))
        attnT_pool = ctx.enter_context(tc.tile_pool(name="attnT", bufs=2))
        small = ctx.enter_context(tc.tile_pool(name="small", bufs=4))
        outsb_pool = ctx.enter_context(tc.tile_pool(name="outsb", bufs=2))
        tp_ps = ctx.enter_context(tc.tile_pool(name="tp_ps", bufs=2, space="PSUM"))
        o_ps = ctx.enter_context(tc.tile_pool(name="o_ps", bufs=2, space="PSUM"))

        def o_proj_piece(Q, attnT, l, oc):
            # one [128, 512] piece of chunk Q's partial o_proj (row-block l)
            si = Q * 4 + l
            po = o_ps.tile([P, QCH], F32, tag="po")
            for h in range(HPG):
                nc.tensor.matmul(
                    po[:], attnT[:, h, l * P:(l + 1) * P],
                    woT_sb[:, h, oc * QCH:(oc + 1) * QCH],
                    start=(h == 0), stop=(h == HPG - 1))
            ob = outsb_pool.tile([P, QCH], F32, tag="osb", name="ob")
            nc.vector.tensor_copy(ob[:], po[:])
            nc.sync.dma_start(
                out=out[si * P:(si + 1) * P, oc * QCH:(oc + 1) * QCH],
                in_=ob[:])

        prev = None  # (Q, attnT) pending o_proj, pipelined one chunk behind
        for Q in range(NQC):
            attnT = attnT_pool.tile([P, HPG, QCH], BF16, tag="attnT")
            for h in range(HPG):
                probs = {}
                for ki in range(NB):
                    cols = [l for l in range(4) if kinds[Q * 4 + l][ki] != "skip"]
                    if not cols:
                        continue
                    lo, hi = min(cols) * P, (max(cols) + 1) * P
                    sc = qk_ps.tile([P, QCH], F32, tag="sc")
                    nc.tensor.matmul(
                        sc[:, lo:hi], krot_sb[:, ki * P:(ki + 1) * P],
                        qrot_sb[:, h, Q * QCH + lo:Q * QCH + hi],
                        start=True, stop=True)
                    for l in cols:
                        kind = kinds[Q * 4 + l][ki]
                        if isinstance(kind, int):
                            nc.vector.tensor_add(
                                sc[:, l * P:(l + 1) * P],
                                sc[:, l * P:(l + 1) * P],
                                mask_sb[:, kind, :])
                    pt = probs_pool.tile([P, QCH], BF16, tag="pt")
                    nc.scalar.activation(
                        out=pt[:, lo:hi], in_=sc[:, lo:hi],
                        func=mybir.ActivationFunctionType.Exp, scale=SCALE)
                    probs[ki] = pt
                for l in range(4):
                    qi = Q * 4 + l
                    kis = [ki for ki in range(NB)
                           if kinds[qi][ki] != "skip" and ki in probs]
                    if not kis:
                        nc.vector.memset(attnT[:, h, l * P:(l + 1) * P], 0.0)
                        if prev is not None:
                            o_proj_piece(prev[0], prev[1], h, l)
                        if Q == NQC - 1 and h == HPG - 1:
                            for oc in range(4):
                                o_proj_piece(Q, attnT, l, oc)
                        continue
                    pv = pv_ps.tile([P, HD + 1], F32, tag="pv")
                    for j, ki in enumerate(kis):
                        nc.tensor.matmul(
                            pv[:], probs[ki][:, l * P:(l + 1) * P],
                            vaug_sb[:, ki, :],
                            start=(j == 0), stop=(j == len(kis) - 1))
                    recip = small.tile([P, 1], F32, tag="recip")
                    nc.vector.reciprocal(recip[:], pv[:, HD:HD + 1])
                    attn = small.tile([P, P], BF16, tag="attn")
                    nc.vector.tensor_scalar_mul(
                        out=attn[:], in0=pv[:, 0:HD], scalar1=recip[:])
                    tp = tp_ps.tile([P, P], BF16, tag="tp")
                    nc.tensor.transpose(tp[:], attn[:], ident[:])
                    nc.scalar.copy(out=attnT[:, h, l * P:(l + 1) * P], in_=tp[:])
                    # o_proj is interleaved into the attention stream so the
                    # PE has dense work while normalize/transpose chains
                    # drain: piece (row h, col l) of the PREVIOUS chunk after
                    # each l; for the last chunk, its own o_proj row right
                    # after the final head completes attnT[:, :, l].
                    if prev is not None:
                        o_proj_piece(prev[0], prev[1], h, l)
                    if Q == NQC - 1 and h == HPG - 1:
                        for oc in range(4):
                            o_proj_piece(Q, attnT, l, oc)
            prev = (Q, attnT)

    nc.compile()
    return nc


_PROGRAM_CACHE = {}


def kernel(x, Wq, Wk, Wv, Wo, cos, sin, attention_mask):
    x = np.asarray(x, dtype=np.float32)
    Wq = np.asarray(Wq, dtype=np.float32)
    Wk = np.asarray(Wk, dtype=np.float32)
    Wv = np.asarray(Wv, dtype=np.float32)
    Wo = np.asarray(Wo, dtype=np.float32)
    cos = np.asarray(cos, dtype=np.float32)
    sin = np.asarray(sin, dtype=np.float32)
    mask = np.asarray(attention_mask, dtype=np.float32)[0, 0]

    kinds, blocks = _classify_mask(mask)
    key = (tuple(tuple(str(k) for k in row) for row in kinds), len(blocks))
    if key not in _PROGRAM_CACHE:
        _PROGRAM_CACHE[key] = _build_program(kinds, len(blocks))
    nc = _PROGRAM_CACHE[key]

    bf = ml_dtypes.bfloat16
    cosT = np.ascontiguousarray(cos[0, 0].T).astype(np.float32)
    sinT = np.ascontiguousarray(sin[0, 0].T).astype(np.float32)
    sinT[0:64] *= -1.0                                   # fold rotate_half sign
    sinP = np.concatenate([sinT[64:], sinT[:64]], axis=0)  # pre-rotate by 64
    maskb = np.stack(blocks, axis=1) if blocks else None   # [P, nblk, P]
    dd = np.arange(P)
    permM = (dd[:, None] == (dd[None, :] + 64) % P).astype(np.float32)

    in_maps = []
    for c in range(N_CORES):
        b, g = c // NKV, c % NKV
        d0, d1 = g * HPG * HD, (g + 1) * HPG * HD
        m = {
            "xT": np.ascontiguousarray(x[b].T).astype(bf),
            "wqT": np.ascontiguousarray(Wq[d0:d1].T).astype(bf),
            "wkT": np.ascontiguousarray(Wk[g * HD:(g + 1) * HD].T).astype(bf),
            "wvT": np.ascontiguousarray(Wv[g * HD:(g + 1) * HD].T).astype(bf),
            "woT": np.ascontiguousarray(Wo[:, d0:d1].T).astype(bf),
            "cosT": cosT.astype(bf),
            "sinP": sinP.astype(bf),
            "perm": permM.astype(bf),
        }
        if maskb is not None:
            m["maskb"] = maskb
        in_maps.append(m)

    global _last_in_maps
    _last_in_maps = in_maps
    res = run_bass_kernel_spmd(nc, in_maps, list(range(N_CORES))).results
    out = np.zeros((B, S, H), np.float32)
    for c in range(N_CORES):
        out[c // NKV] += res[c]["out"]
    return out
